# revision 2
# baseline (speedup 1.0000x reference)
"""Multi-head attention Trainium2 kernel, 8-core SPMD (v6).

Problem: x[2,4096,512], 8 heads of 64; per-head QKV proj, softmax(QK^T/8)V,
concat, output proj.

Sharding: sequence-parallel, no collectives. Core c handles batch b=c//4 and
query rows [1024*(c%4), ...+1024). Each core computes K/V for the full 4096-row
sequence of its batch; x is host-rolled so local query rows sit at 0:1024.

v4 structure:
- bk dropped (softmax-invariant); bv folded into bo host-side.
- PSUM: shared 3-deep [128,2,512] ring for scores AND per-eg projection
  microtasks + 2 PV accumulators = 8 banks. Fine proj tasks retire ring
  slots quickly (convert ~0.6us vs exp ~1.1us), keeping recycling fast.
- projection emitted as per-eg microtasks (4 matmuls + 1 convert) popped
  between score pairs, so PE never runs >1us of projection while the exp
  engines starve.
- exp lanes near-alternate ACT/DVE (weighted 0.54 toward the faster ACT)
  so consecutive score slots retire on different engines; converts and
  drains are assigned by an ns-accurate load balancer.
- normalize: DVE reciprocal + Pool partition_broadcast + Pool multiply
  (no DRAM round-trip).
- group A ramps per availability; full-16 blocks skip the yacc merge.
- V projection in fp8 DoubleRow with a host-side weight-residual
  (Wv ~ W8 + fp8(Wv-W8)): 4 DR matmuls of contraction 256 instead of 4
  bf16 matmuls of contraction 128 -> half the PE time; error stays at
  the x-fp8-quantization level which the PV averaging washes out.
"""

import numpy as np

import concourse.bass as bass
from concourse import bacc
import concourse.mybir as mybir
import concourse.tile as tile
from concourse.bass_utils import run_bass_kernel_spmd

F32 = mybir.dt.float32
F32R = mybir.dt.float32r
BF16 = mybir.dt.bfloat16
FP8 = mybir.dt.float8e4
I8 = mybir.dt.int8
DR = mybir.MatmulPerfMode.DoubleRow
ADD = mybir.AluOpType.add
MULT = mybir.AluOpType.mult
EXPF = mybir.ActivationFunctionType.Exp
IDENT = mybir.ActivationFunctionType.Identity

B, S, D, H, E = 2, 4096, 512, 8, 64
NCORES = 8
QCHUNK = S // 4
TCH = 512
NPAIR = S // 256

A_SCH = 1.442695041      # 8*log2(e)*0.125
B_SCH = 55.632

import os
A_CNT = [2, 2, 4, 4, 6, 6, 8, 8, 10, 10, 12, 12, 14, 14, 16, 16]
LAG = int(os.environ.get("K_LAG", "5"))
PPOP = int(os.environ.get("K_PPOP", "2"))
EXP_FRAC_A = float(os.environ.get("K_EFA", "0.54"))
D_PRELOAD = float(os.environ.get("K_DPRE", "0"))


def build_program():
    nc = bacc.Bacc()
    xt_d = nc.dram_tensor("xt", [D, S], BF16, kind="ExternalInput")
    wq_d = nc.dram_tensor("wq", [128, 4, 512], BF16, kind="ExternalInput")
    wk_d = nc.dram_tensor("wk", [128, 4, 512], BF16, kind="ExternalInput")
    x8_d = nc.dram_tensor("x8", [128, 2, 2, S], FP8, kind="ExternalInput")
    wv_d = nc.dram_tensor("wv", [128, 2, 2, 512], FP8, kind="ExternalInput")
    wvr_d = nc.dram_tensor("wvr", [128, 2, 2, 512], FP8, kind="ExternalInput")
    wo_d = nc.dram_tensor("wo", [64, 8, 512], F32R, kind="ExternalInput")
    bq_d = nc.dram_tensor("bq", [128, 4], F32, kind="ExternalInput")
    bo_d = nc.dram_tensor("bo", [512], F32, kind="ExternalInput")
    out_d = nc.dram_tensor("out", [QCHUNK, D], F32, kind="ExternalOutput")

    with tile.TileContext(nc) as tc:
        with (
            tc.tile_pool(name="const", bufs=1) as cpool,
            tc.tile_pool(name="work", bufs=4) as wpool,
            tc.tile_pool(name="misc", bufs=3) as mpool,
            tc.tile_pool(name="xtp", bufs=3) as xpool,
            tc.tile_pool(name="sring", bufs=3, space="PSUM") as spool,
            tc.tile_pool(name="pav", bufs=2, space="PSUM") as pavpool,
        ):
            wq_s = cpool.tile([128, 4, 512], BF16, tag="wq")
            wk_s = cpool.tile([128, 4, 512], BF16, tag="wk")
            x8_s = cpool.tile([128, 2, 2, S], FP8, tag="x8")
            wv_s = cpool.tile([128, 2, 2, 512], FP8, tag="wv")
            wvr_s = cpool.tile([128, 2, 2, 512], FP8, tag="wvr")
            wo_s = cpool.tile([64, 8, 512], F32R, tag="wo")
            bq_s = cpool.tile([128, 4], F32, tag="bq")
            bo_r = cpool.tile([128, 512], F32, tag="bor")

            kT8 = cpool.tile([128, 4, S], FP8, tag="kT8")
            qT8 = cpool.tile([128, 4, QCHUNK], FP8, tag="qT8")
            vA = cpool.tile([128, S // 128, H, E + 2], FP8, tag="vA")
            yacc = cpool.tile([65, 16, 512], F32, tag="yacc")
            yT2 = cpool.tile([64, H, QCHUNK], F32R, tag="yT2")

            nc.sync.dma_start(wk_s[:], wk_d[:])
            nc.gpsimd.memset(vA[:, :, :, E], 1.0)
            nc.gpsimd.memset(vA[:, :, :, E + 1], 0.0)

            # --- engine load balancing ------------------------------------
            load = {"A": 0.0, "D": D_PRELOAD}
            expn = {"A": 0, "D": 0}

            def pick():
                return "A" if load["A"] <= load["D"] else "D"

            def exp_lane():
                tot = expn["A"] + expn["D"]
                lane = "A" if (tot == 0 or
                               expn["A"] < EXP_FRAC_A * (tot + 1)) else "D"
                expn[lane] += 1
                return lane

            def act_ns(free):
                return free * 0.8333 + 185.0

            def dve_ns(free):
                return free * 1.0417 + 125.0

            # --- projection microtasks ------------------------------------
            xstate = {}
            chunks_done = [0]

            def t_x(ch):
                def f():
                    xT = xpool.tile([128, 4, TCH], BF16, tag="xT")
                    for ds in range(4):
                        nc.sync.dma_start(
                            xT[:, ds, :],
                            xt_d[ds * 128:(ds + 1) * 128,
                                 ch * TCH:(ch + 1) * TCH])
                    xstate[ch] = xT
                return f

            def t_k(ch, eg):
                def f():
                    xT = xstate[ch]
                    St = spool.tile([128, 2, 512], F32, tag="S")
                    Pt = St[:, 0, :]
                    for ds in range(4):
                        nc.tensor.matmul(
                            Pt, wk_s[:, ds, eg * 128:(eg + 1) * 128],
                            xT[:, ds, :], start=(ds == 0), stop=(ds == 3))
                    dst = kT8[:, eg, ch * TCH:(ch + 1) * TCH]
                    if pick() == "A":
                        nc.scalar.activation(dst, Pt, IDENT, scale=1.0)
                        load["A"] += act_ns(512)
                    else:
                        nc.vector.tensor_copy(dst, Pt)
                        load["D"] += dve_ns(512)
                return f

            def t_q(ch, eg):
                def f():
                    xT = xstate[ch]
                    St = spool.tile([128, 2, 512], F32, tag="S")
                    Pt = St[:, 0, :]
                    for ds in range(4):
                        nc.tensor.matmul(
                            Pt, wq_s[:, ds, eg * 128:(eg + 1) * 128],
                            xT[:, ds, :], start=(ds == 0), stop=(ds == 3))
                    dst = qT8[:, eg, ch * TCH:(ch + 1) * TCH]
                    if pick() == "A":
                        nc.scalar.activation(dst, Pt, IDENT,
                                             bias=bq_s[:, eg:eg + 1],
                                             scale=1.0)
                        load["A"] += act_ns(512)
                    else:
                        nc.vector.tensor_scalar(dst, Pt,
                                                bq_s[:, eg:eg + 1], None, ADD)
                        load["D"] += dve_ns(512)
                return f

            def t_v(ch, ts):
                def f():
                    St = spool.tile([128, 2, 512], F32, tag="S")
                    Pt = St[:, 0, :]
                    t0 = ch * TCH + ts * 128
                    step = 0
                    for wt in (wv_s, wvr_s):
                        for ds2 in range(2):
                            nc.tensor.matmul(
                                Pt, x8_s[:, ds2, :, t0:t0 + 128],
                                wt[:, ds2, :, :], start=(step == 0),
                                stop=(step == 3), perf_mode=DR)
                            step += 1
                    dst = vA[:, ch * 4 + ts, :, 0:E]
                    src = Pt.rearrange("p (h e) -> p h e", h=H)
                    if pick() == "A":
                        nc.scalar.activation(dst, src, IDENT, scale=1.0)
                        load["A"] += act_ns(512)
                    else:
                        nc.vector.tensor_copy(dst, src)
                        load["D"] += dve_ns(512)
                    if ts == 3:
                        chunks_done[0] = max(chunks_done[0], ch + 1)
                return f

            def chunk_tasks(ch):
                ts = [t_k(ch, 0), t_k(ch, 1)]
                if ch < 2:
                    ts += [t_q(ch, 0), t_q(ch, 1)]
                ts += [t_k(ch, 2), t_k(ch, 3)]
                if ch < 2:
                    ts += [t_q(ch, 2), t_q(ch, 3)]
                ts += [t_v(ch, 0), t_v(ch, 1), t_v(ch, 2), t_v(ch, 3)]
                return ts

            proj_queue = []
            pop_ctr = [0]

            def pop_proj(n=1):
                for _ in range(n):
                    if proj_queue:
                        proj_queue.pop(0)()

            def need_chunk(c):
                while chunks_done[0] <= c and proj_queue:
                    pop_proj(1)

            # --- attention block ------------------------------------------
            def emit_block(h, sc, pairs, mode):
                a0 = 32 * (h % 4)
                g0 = 2 * (h // 4)
                n = len(pairs)
                pav = pavpool.tile([128, 512], F32, tag="pav")
                pend = []

                def emit_pv(item):
                    j, tp, ptv = item
                    nc.tensor.matmul(
                        pav[0:66, :], vA[:, 2 * tp:2 * tp + 2, h, :], ptv,
                        start=(j == 0), stop=(j == n - 1), perf_mode=DR)

                for i in range(n):
                    tp = pairs[i]
                    need_chunk(tp // 2)
                    St = spool.tile([128, 2, 512], F32, tag="S")
                    for kt in (0, 1):
                        tt = 2 * tp + kt
                        nc.tensor.matmul(
                            St[:, kt, :],
                            kT8[a0:a0 + 32, g0:g0 + 2,
                                tt * 128:(tt + 1) * 128],
                            qT8[a0:a0 + 32, g0:g0 + 2,
                                sc * 512:(sc + 1) * 512],
                            start=True, stop=True, perf_mode=DR,
                            tile_position=(a0, 0))
                    lane = pick()
                    if lane == "A":
                        pt = wpool.tile([128, 2, 512], FP8, tag="ptA")
                        nc.scalar.activation(pt[:], St[:], EXPF, scale=0.125)
                        load["A"] += act_ns(1024)
                        ptv = pt[:]
                    else:
                        pti = wpool.tile([128, 2, 512], I8, tag="ptD")
                        nc.vector.tensor_scalar(
                            pti[:], St[:], A_SCH, B_SCH, MULT, ADD)
                        load["D"] += dve_ns(1024)
                        ptv = pti[:].bitcast(FP8)
                    pend.append((i, tp, ptv))
                    pop_ctr[0] += 1
                    if pop_ctr[0] >= PPOP:
                        pop_ctr[0] = 0
                        pop_proj(1)
                    while len(pend) > LAG:
                        emit_pv(pend.pop(0))
                for item in pend:
                    emit_pv(item)

                slot = sc * 8 + h
                if mode == "partA":
                    if pick() == "A":
                        nc.scalar.activation(yacc[:, slot, :], pav[0:65, :],
                                             IDENT, scale=1.0)
                        load["A"] += act_ns(512)
                    else:
                        nc.vector.tensor_copy(yacc[:, slot, :], pav[0:65, :])
                        load["D"] += dve_ns(512)
                    return
                if mode == "full":
                    tmp = mpool.tile([65, 512], F32, tag="tmp")
                    if pick() == "A":
                        nc.scalar.activation(tmp[:], pav[0:65, :], IDENT,
                                             scale=1.0)
                        load["A"] += act_ns(512)
                    else:
                        nc.vector.tensor_copy(tmp[:], pav[0:65, :])
                        load["D"] += dve_ns(512)
                else:  # "B"
                    tmp = mpool.tile([65, 512], F32, tag="tmp")
                    nc.vector.tensor_tensor(
                        out=tmp[:], in0=yacc[:, slot, :], in1=pav[0:65, :],
                        op=ADD)
                    load["D"] += dve_ns(512)
                rec = mpool.tile([1, 512], F32, tag="rec")
                nc.vector.reciprocal(rec[:], tmp[64:65, :])
                load["D"] += dve_ns(512)
                den = mpool.tile([64, 512], F32, tag="den")
                nc.gpsimd.partition_broadcast(den[:], rec[:])
                nc.gpsimd.tensor_tensor(
                    out=yT2[0:64, h, sc * 512:(sc + 1) * 512],
                    in0=tmp[0:64, :], in1=den[:], op=MULT)

            # --- output projection ----------------------------------------
            def phase3_task(sc, sta):
                def f():
                    for i in range(2):
                        st = sc * 4 + sta * 2 + i
                        po = pavpool.tile([128, 512], F32, tag="pav")
                        for h in range(H):
                            nc.tensor.matmul(
                                po[:],
                                yT2[0:64, h, st * 128:(st + 1) * 128],
                                wo_s[0:64, h, :], start=(h == 0),
                                stop=(h == 7))
                        o_s = mpool.tile([128, 512], F32, tag="osb")
                        nc.vector.tensor_tensor(out=o_s[:], in0=po[:],
                                                in1=bo_r[:], op=ADD)
                        load["D"] += dve_ns(512)
                        nc.sync.dma_start(out_d[st * 128:(st + 1) * 128, :],
                                          o_s[:])
                return f

            # --- emission --------------------------------------------------
            t_x(0)()
            t_k(0, 0)()
            nc.sync.dma_start(wq_s[:], wq_d[:])
            nc.sync.dma_start(bq_s[:], bq_d[:])
            t_k(0, 1)()
            t_q(0, 0)()
            t_q(0, 1)()
            nc.sync.dma_start(wv_s[:], wv_d[:])
            nc.sync.dma_start(wvr_s[:], wvr_d[:])
            nc.sync.dma_start(x8_s[:], x8_d[:])
            proj_queue.extend([t_k(0, 2), t_k(0, 3), t_q(0, 2), t_q(0, 3),
                               t_v(0, 0), t_v(0, 1), t_v(0, 2), t_v(0, 3)])
            for ch in range(1, 8):
                ts = chunk_tasks(ch)
                # promote the x DMA ahead of the previous chunk's tail
                ins = max(0, len(proj_queue) - 6)
                proj_queue.insert(ins, t_x(ch))
                proj_queue.extend(ts)

            blocks = [(sc, h) for sc in (0, 1) for h in range(H)]
            for bi, (sc, h) in enumerate(blocks):
                a = A_CNT[bi]
                mode = "full" if a == NPAIR else "partA"
                emit_block(h, sc, list(range(a)), mode)
                if bi == 11:
                    nc.sync.dma_start(
                        bo_r[:], bo_d[:].unsqueeze(0).to_broadcast((128, 512)))
                    nc.sync.dma_start(wo_s[:], wo_d[:])
                pop_proj(1)
            pop_proj(len(proj_queue))

            # group B: sc1 remainders first, then sc0 with p3(sc1) spread in
            for h in range(H):
                bi = 8 + h
                if A_CNT[bi] < NPAIR:
                    emit_block(h, 1, list(range(A_CNT[bi], NPAIR)), "B")
            p3s = [phase3_task(1, 0), phase3_task(1, 1)]
            for h in range(H):
                emit_block(h, 0, list(range(A_CNT[h], NPAIR)), "B")
                if p3s and h in (2, 5):
                    p3s.pop(0)()
            for t in p3s:
                t()
            phase3_task(0, 0)()
            phase3_task(0, 1)()
    nc.compile()
    return nc


_NC = None


def _pack_weights(Wq, bq, Wk, bk, Wv, bv, Wo, bo):
    import ml_dtypes
    s = lambda a: np.ascontiguousarray(np.asarray(a, np.float32))
    sb = lambda a: np.ascontiguousarray(
        np.asarray(a, np.float32).astype(ml_dtypes.bfloat16))
    # e-permutation for DoubleRow plane layout: column c = eg*128+p of the
    # stationary maps to head h = p//32 + 4*(eg//2), e = 32*(eg%2) + p%32
    p = np.arange(128)
    eg = np.arange(4)
    hh = p[None, :] // 32 + 4 * (eg[:, None] // 2)     # [4,128]
    ee = 32 * (eg[:, None] % 2) + p[None, :] % 32      # [4,128]

    def pack_qk(W):
        t = np.asarray(W, np.float32)[hh, :, ee]       # [4,128,512(d)]
        t = t.transpose(2, 0, 1)                       # [d, eg, p]
        t = t.reshape(4, 128, 4, 128)                  # [ds, pd, eg, p]
        return sb(t.transpose(1, 0, 2, 3).reshape(128, 4, 512))

    def pack_b(b):
        return s(np.asarray(b, np.float32)[hh, ee].T)  # [128,4]

    wq_p = pack_qk(Wq)
    wk_p = pack_qk(Wk)   # bk dropped: softmax-invariant
    bq_p = pack_b(bq)
    f8 = lambda a: np.asarray(a, np.float32).astype(ml_dtypes.float8_e4m3)
    wv_flat = np.transpose(np.asarray(Wv, np.float32), (1, 0, 2)).reshape(D, 512)
    wv8 = f8(wv_flat)
    wvr8 = f8(wv_flat - wv8.astype(np.float32))
    # [d, e] -> [pd, ds2, pl, e] with d = ds2*256 + pl*128 + pd
    pk_v = lambda w: np.ascontiguousarray(
        w.reshape(2, 2, 128, 512).transpose(2, 0, 1, 3))
    wv_p = pk_v(wv8)
    wvr_p = pk_v(wvr8)
    wo_p = s(np.asarray(Wo, np.float32).reshape(8, 64, 512).transpose(1, 0, 2))
    # bv folded into bo: cat(y_h + bv_h) @ Wo + bo = cat(y_h) @ Wo + bo'
    bo_p = s(np.asarray(bo, np.float32) +
             np.asarray(bv, np.float32).reshape(H * E) @ np.asarray(Wo, np.float32))
    return dict(wq=wq_p, wk=wk_p, wv=wv_p, wvr=wvr_p, wo=wo_p, bq=bq_p,
                bo=bo_p)


def kernel(x, Wq, bq, Wk, bk, Wv, bv, Wo, bo, **kw):
    global _NC
    x = np.asarray(x, np.float32)
    packed = _pack_weights(Wq, bq, Wk, bk, Wv, bv, Wo, bo)

    if _NC is None:
        _NC = build_program()

    in_maps = []
    for c in range(NCORES):
        b = c // 4
        q0 = (c % 4) * QCHUNK
        xb = np.roll(x[b], -q0, axis=0)  # queries at rows 0:1024
        import ml_dtypes
        xbT = xb.T  # [d, t]
        m = {"xt": np.ascontiguousarray(xbT.astype(ml_dtypes.bfloat16)),
             "x8": np.ascontiguousarray(
                 xbT.astype(ml_dtypes.float8_e4m3)
                 .reshape(2, 2, 128, S).transpose(2, 0, 1, 3))}
        m.update(packed)
        in_maps.append(m)
    res = run_bass_kernel_spmd(_NC, in_maps, core_ids=list(range(NCORES)))
    out = np.empty((B, S, D), np.float32)
    for c in range(NCORES):
        b = c // 4
        q0 = (c % 4) * QCHUNK
        out[b, q0:q0 + QCHUNK] = res.results[c]["out"]
    return out


# revision 8
# speedup vs baseline: 1.0209x; 1.0209x over previous
"""Multi-head attention Trainium2 kernel, 8-core SPMD (v6).

Problem: x[2,4096,512], 8 heads of 64; per-head QKV proj, softmax(QK^T/8)V,
concat, output proj.

Sharding: sequence-parallel, no collectives. Core c handles batch b=c//4 and
query rows [1024*(c%4), ...+1024). Each core computes K/V for the full 4096-row
sequence of its batch; x is host-rolled so local query rows sit at 0:1024.

v4 structure:
- bk dropped (softmax-invariant); bv folded into bo host-side.
- PSUM: shared 3-deep [128,2,512] ring for scores AND per-eg projection
  microtasks + 2 PV accumulators = 8 banks. Fine proj tasks retire ring
  slots quickly (convert ~0.6us vs exp ~1.1us), keeping recycling fast.
- projection emitted as per-eg microtasks (4 matmuls + 1 convert) popped
  between score pairs, so PE never runs >1us of projection while the exp
  engines starve.
- exp lanes near-alternate ACT/DVE (weighted 0.54 toward the faster ACT)
  so consecutive score slots retire on different engines; converts and
  drains are assigned by an ns-accurate load balancer.
- normalize: DVE reciprocal + Pool partition_broadcast + Pool multiply
  (no DRAM round-trip).
- group A ramps per availability; full-16 blocks skip the yacc merge.
- V projection in fp8 DoubleRow with a host-side weight-residual
  (Wv ~ W8 + fp8(Wv-W8)): 4 DR matmuls of contraction 256 instead of 4
  bf16 matmuls of contraction 128 -> half the PE time; error stays at
  the x-fp8-quantization level which the PV averaging washes out.
"""

import numpy as np

import concourse.bass as bass
from concourse import bacc
import concourse.mybir as mybir
import concourse.tile as tile
from concourse.bass_utils import run_bass_kernel_spmd

F32 = mybir.dt.float32
F32R = mybir.dt.float32r
BF16 = mybir.dt.bfloat16
FP8 = mybir.dt.float8e4
I8 = mybir.dt.int8
DR = mybir.MatmulPerfMode.DoubleRow
ADD = mybir.AluOpType.add
MULT = mybir.AluOpType.mult
EXPF = mybir.ActivationFunctionType.Exp
IDENT = mybir.ActivationFunctionType.Identity

B, S, D, H, E = 2, 4096, 512, 8, 64
NCORES = 8
QCHUNK = S // 4
TCH = 512
NPAIR = S // 256

A_SCH = 1.442695041      # 8*log2(e)*0.125
B_SCH = 55.632

A_CNT = [2, 2, 4, 4, 6, 8, 10, 12, 14, 16, 16, 16, 16, 16, 16, 16]
LAG = 5                  # pairs of exp->PV lag
PPOP = 2                 # pop 1 proj microtask every PPOP pairs
D_PRELOAD = 0.0          # initial DVE virtual load


def build_program():
    nc = bacc.Bacc()
    xt_d = nc.dram_tensor("xt", [D, S], BF16, kind="ExternalInput")
    wq_d = nc.dram_tensor("wq", [128, 4, 512], BF16, kind="ExternalInput")
    wk_d = nc.dram_tensor("wk", [128, 4, 512], BF16, kind="ExternalInput")
    x8_d = nc.dram_tensor("x8", [128, 2, 2, S], FP8, kind="ExternalInput")
    wv_d = nc.dram_tensor("wv", [128, 2, 2, 512], FP8, kind="ExternalInput")
    wvr_d = nc.dram_tensor("wvr", [128, 2, 2, 512], FP8, kind="ExternalInput")
    wo_d = nc.dram_tensor("wo", [64, 8, 512], F32R, kind="ExternalInput")
    bq_d = nc.dram_tensor("bq", [128, 4], F32, kind="ExternalInput")
    bo_d = nc.dram_tensor("bo", [512], F32, kind="ExternalInput")
    out_d = nc.dram_tensor("out", [QCHUNK, D], F32, kind="ExternalOutput")

    with tile.TileContext(nc) as tc:
        with (
            tc.tile_pool(name="const", bufs=1) as cpool,
            tc.tile_pool(name="work", bufs=6) as wpool,
            tc.tile_pool(name="misc", bufs=3) as mpool,
            tc.tile_pool(name="xtp", bufs=3) as xpool,
            tc.tile_pool(name="sring", bufs=6, space="PSUM") as spool,
            tc.tile_pool(name="pav", bufs=2, space="PSUM") as pavpool,
        ):
            wq_s = cpool.tile([128, 4, 512], BF16, tag="wq")
            wk_s = cpool.tile([128, 4, 512], BF16, tag="wk")
            x8_s = cpool.tile([128, 2, 2, S], FP8, tag="x8")
            wv_s = cpool.tile([128, 2, 2, 512], FP8, tag="wv")
            wvr_s = cpool.tile([128, 2, 2, 512], FP8, tag="wvr")
            wo_s = cpool.tile([64, 8, 512], F32R, tag="wo")
            bq_s = cpool.tile([128, 4], F32, tag="bq")
            bo_r = cpool.tile([128, 512], F32, tag="bor")

            kT8 = cpool.tile([128, 4, S], FP8, tag="kT8")
            qT8 = cpool.tile([128, 4, QCHUNK], FP8, tag="qT8")
            vA = cpool.tile([128, S // 128, H, E + 2], FP8, tag="vA")
            yacc = cpool.tile([65, 16, 512], F32, tag="yacc")
            yT2 = cpool.tile([64, H, QCHUNK], F32R, tag="yT2")

            nc.sync.dma_start(wk_s[:], wk_d[:])
            nc.gpsimd.memset(vA[:, :, :, E], 1.0)
            nc.gpsimd.memset(vA[:, :, :, E + 1], 0.0)

            # --- engine load balancing ------------------------------------
            load = {"A": 0.0, "D": D_PRELOAD}

            def pick():
                return "A" if load["A"] <= load["D"] else "D"

            def act_ns(free):
                return free * 0.8333 + 185.0

            def dve_ns(free):
                return free * 1.0417 + 125.0

            # --- projection microtasks ------------------------------------
            xstate = {}
            chunks_done = [0]

            def t_x(ch):
                def f():
                    xT = xpool.tile([128, 4, TCH], BF16, tag="xT")
                    for ds in range(4):
                        nc.sync.dma_start(
                            xT[:, ds, :],
                            xt_d[ds * 128:(ds + 1) * 128,
                                 ch * TCH:(ch + 1) * TCH])
                    xstate[ch] = xT
                return f

            def t_k(ch, eg):
                def f():
                    xT = xstate[ch]
                    Pt = spool.tile([128, 512], F32, tag="S")
                    for ds in range(4):
                        nc.tensor.matmul(
                            Pt[:], wk_s[:, ds, eg * 128:(eg + 1) * 128],
                            xT[:, ds, :], start=(ds == 0), stop=(ds == 3))
                    dst = kT8[:, eg, ch * TCH:(ch + 1) * TCH]
                    if pick() == "A":
                        nc.scalar.activation(dst, Pt[:], IDENT, scale=1.0)
                        load["A"] += act_ns(512)
                    else:
                        nc.vector.tensor_copy(dst, Pt[:])
                        load["D"] += dve_ns(512)
                return f

            def t_q(ch, eg):
                def f():
                    xT = xstate[ch]
                    Pt = spool.tile([128, 512], F32, tag="S")
                    for ds in range(4):
                        nc.tensor.matmul(
                            Pt[:], wq_s[:, ds, eg * 128:(eg + 1) * 128],
                            xT[:, ds, :], start=(ds == 0), stop=(ds == 3))
                    dst = qT8[:, eg, ch * TCH:(ch + 1) * TCH]
                    if pick() == "A":
                        nc.scalar.activation(dst, Pt[:], IDENT,
                                             bias=bq_s[:, eg:eg + 1],
                                             scale=1.0)
                        load["A"] += act_ns(512)
                    else:
                        nc.vector.tensor_scalar(dst, Pt[:],
                                                bq_s[:, eg:eg + 1], None, ADD)
                        load["D"] += dve_ns(512)
                return f

            def t_v(ch, ts):
                def f():
                    Pt = spool.tile([128, 512], F32, tag="S")
                    t0 = ch * TCH + ts * 128
                    step = 0
                    for wt in (wv_s, wvr_s):
                        for ds2 in range(2):
                            nc.tensor.matmul(
                                Pt[:], x8_s[:, ds2, :, t0:t0 + 128],
                                wt[:, ds2, :, :], start=(step == 0),
                                stop=(step == 3), perf_mode=DR)
                            step += 1
                    dst = vA[:, ch * 4 + ts, :, 0:E]
                    src = Pt[:].rearrange("p (h e) -> p h e", h=H)
                    if pick() == "A":
                        nc.scalar.activation(dst, src, IDENT, scale=1.0)
                        load["A"] += act_ns(512)
                    else:
                        nc.vector.tensor_copy(dst, src)
                        load["D"] += dve_ns(512)
                    if ts == 3:
                        chunks_done[0] = max(chunks_done[0], ch + 1)
                return f

            def chunk_tasks(ch):
                ts = [t_k(ch, 0), t_k(ch, 1)]
                if ch < 2:
                    ts += [t_q(ch, 0), t_q(ch, 1)]
                ts += [t_k(ch, 2), t_k(ch, 3)]
                if ch < 2:
                    ts += [t_q(ch, 2), t_q(ch, 3)]
                ts += [t_v(ch, 0), t_v(ch, 1), t_v(ch, 2), t_v(ch, 3)]
                return ts

            proj_queue = []
            pop_ctr = [0]

            def pop_proj(n=1):
                for _ in range(n):
                    if proj_queue:
                        proj_queue.pop(0)()

            def need_chunk(c):
                while chunks_done[0] <= c and proj_queue:
                    pop_proj(1)

            # --- attention block ------------------------------------------
            def emit_block(h, sc, pairs, mode):
                a0 = 32 * (h % 4)
                g0 = 2 * (h // 4)
                n = len(pairs)
                pav = pavpool.tile([128, 512], F32, tag="pav")
                pend = []

                def emit_pv(item):
                    j, tp, ptv = item
                    nc.tensor.matmul(
                        pav[0:66, :], vA[:, 2 * tp:2 * tp + 2, h, :], ptv,
                        start=(j == 0), stop=(j == n - 1), perf_mode=DR)

                for i in range(n):
                    tp = pairs[i]
                    need_chunk(tp // 2)
                    ptp = wpool.tile([128, 2, 512], I8, tag="ptD")
                    for kt in (0, 1):
                        tt = 2 * tp + kt
                        Sk = spool.tile([128, 512], F32, tag="S")
                        nc.tensor.matmul(
                            Sk[:],
                            kT8[a0:a0 + 32, g0:g0 + 2,
                                tt * 128:(tt + 1) * 128],
                            qT8[a0:a0 + 32, g0:g0 + 2,
                                sc * 512:(sc + 1) * 512],
                            start=True, stop=True, perf_mode=DR,
                            tile_position=(a0, 0))
                        if pick() == "A":
                            nc.scalar.activation(
                                ptp[:, kt, :].bitcast(FP8), Sk[:], EXPF,
                                scale=0.125)
                            load["A"] += act_ns(512)
                        else:
                            nc.vector.tensor_scalar(
                                ptp[:, kt, :], Sk[:], A_SCH, B_SCH, MULT,
                                ADD)
                            load["D"] += dve_ns(512)
                    ptv = ptp[:].bitcast(FP8)
                    pend.append((i, tp, ptv))
                    pop_ctr[0] += 1
                    if pop_ctr[0] >= PPOP:
                        pop_ctr[0] = 0
                        pop_proj(1)
                    while len(pend) > LAG:
                        emit_pv(pend.pop(0))
                for item in pend:
                    emit_pv(item)

                slot = sc * 8 + h
                if mode == "partA":
                    if pick() == "A":
                        nc.scalar.activation(yacc[:, slot, :], pav[0:65, :],
                                             IDENT, scale=1.0)
                        load["A"] += act_ns(512)
                    else:
                        nc.vector.tensor_copy(yacc[:, slot, :], pav[0:65, :])
                        load["D"] += dve_ns(512)
                    return
                if mode == "full":
                    tmp = mpool.tile([65, 512], F32, tag="tmp")
                    if pick() == "A":
                        nc.scalar.activation(tmp[:], pav[0:65, :], IDENT,
                                             scale=1.0)
                        load["A"] += act_ns(512)
                    else:
                        nc.vector.tensor_copy(tmp[:], pav[0:65, :])
                        load["D"] += dve_ns(512)
                else:  # "B"
                    tmp = mpool.tile([65, 512], F32, tag="tmp")
                    nc.vector.tensor_tensor(
                        out=tmp[:], in0=yacc[:, slot, :], in1=pav[0:65, :],
                        op=ADD)
                    load["D"] += dve_ns(512)
                rec = mpool.tile([1, 512], F32, tag="rec")
                nc.vector.reciprocal(rec[:], tmp[64:65, :])
                load["D"] += dve_ns(512)
                den = mpool.tile([64, 512], F32, tag="den")
                nc.gpsimd.partition_broadcast(den[:], rec[:])
                nc.gpsimd.tensor_tensor(
                    out=yT2[0:64, h, sc * 512:(sc + 1) * 512],
                    in0=tmp[0:64, :], in1=den[:], op=MULT)

            # --- output projection ----------------------------------------
            def phase3_task(sc, sta):
                def f():
                    for i in range(2):
                        st = sc * 4 + sta * 2 + i
                        po = pavpool.tile([128, 512], F32, tag="pav")
                        for h in range(H):
                            nc.tensor.matmul(
                                po[:],
                                yT2[0:64, h, st * 128:(st + 1) * 128],
                                wo_s[0:64, h, :], start=(h == 0),
                                stop=(h == 7))
                        o_s = mpool.tile([128, 512], F32, tag="osb")
                        nc.vector.tensor_tensor(out=o_s[:], in0=po[:],
                                                in1=bo_r[:], op=ADD)
                        load["D"] += dve_ns(512)
                        nc.sync.dma_start(out_d[st * 128:(st + 1) * 128, :],
                                          o_s[:])
                return f

            # --- emission --------------------------------------------------
            t_x(0)()
            t_k(0, 0)()
            nc.sync.dma_start(wq_s[:], wq_d[:])
            nc.sync.dma_start(bq_s[:], bq_d[:])
            t_k(0, 1)()
            t_q(0, 0)()
            t_q(0, 1)()
            nc.sync.dma_start(wv_s[:], wv_d[:])
            nc.sync.dma_start(wvr_s[:], wvr_d[:])
            nc.sync.dma_start(x8_s[:], x8_d[:])
            proj_queue.extend([t_k(0, 2), t_k(0, 3), t_q(0, 2), t_q(0, 3),
                               t_v(0, 0), t_v(0, 1), t_v(0, 2), t_v(0, 3)])
            for ch in range(1, 8):
                ts = chunk_tasks(ch)
                # promote the x DMA ahead of the previous chunk's tail
                ins = max(0, len(proj_queue) - 6)
                proj_queue.insert(ins, t_x(ch))
                proj_queue.extend(ts)

            blocks = [(sc, h) for sc in (0, 1) for h in range(H)]
            for bi, (sc, h) in enumerate(blocks):
                a = A_CNT[bi]
                mode = "full" if a == NPAIR else "partA"
                emit_block(h, sc, list(range(a)), mode)
                if bi == 11:
                    nc.sync.dma_start(
                        bo_r[:], bo_d[:].unsqueeze(0).to_broadcast((128, 512)))
                    nc.sync.dma_start(wo_s[:], wo_d[:])
                pop_proj(1)
            pop_proj(len(proj_queue))

            # group B: sc1 remainders first, then sc0 with p3(sc1) spread in
            for h in range(H):
                bi = 8 + h
                if A_CNT[bi] < NPAIR:
                    emit_block(h, 1, list(range(A_CNT[bi], NPAIR)), "B")
            p3s = [phase3_task(1, 0), phase3_task(1, 1)]
            for h in range(H):
                emit_block(h, 0, list(range(A_CNT[h], NPAIR)), "B")
                if p3s and h in (1, 3):
                    p3s.pop(0)()
            for t in p3s:
                t()
            phase3_task(0, 0)()
            phase3_task(0, 1)()
    nc.compile()
    return nc


_NC = None


def _pack_weights(Wq, bq, Wk, bk, Wv, bv, Wo, bo):
    import ml_dtypes
    s = lambda a: np.ascontiguousarray(np.asarray(a, np.float32))
    sb = lambda a: np.ascontiguousarray(
        np.asarray(a, np.float32).astype(ml_dtypes.bfloat16))
    # e-permutation for DoubleRow plane layout: column c = eg*128+p of the
    # stationary maps to head h = p//32 + 4*(eg//2), e = 32*(eg%2) + p%32
    p = np.arange(128)
    eg = np.arange(4)
    hh = p[None, :] // 32 + 4 * (eg[:, None] // 2)     # [4,128]
    ee = 32 * (eg[:, None] % 2) + p[None, :] % 32      # [4,128]

    def pack_qk(W):
        t = np.asarray(W, np.float32)[hh, :, ee]       # [4,128,512(d)]
        t = t.transpose(2, 0, 1)                       # [d, eg, p]
        t = t.reshape(4, 128, 4, 128)                  # [ds, pd, eg, p]
        return sb(t.transpose(1, 0, 2, 3).reshape(128, 4, 512))

    def pack_b(b):
        return s(np.asarray(b, np.float32)[hh, ee].T)  # [128,4]

    wq_p = pack_qk(Wq)
    wk_p = pack_qk(Wk)   # bk dropped: softmax-invariant
    bq_p = pack_b(bq)
    f8 = lambda a: np.asarray(a, np.float32).astype(ml_dtypes.float8_e4m3)
    wv_flat = np.transpose(np.asarray(Wv, np.float32), (1, 0, 2)).reshape(D, 512)
    wv8 = f8(wv_flat)
    wvr8 = f8(wv_flat - wv8.astype(np.float32))
    # [d, e] -> [pd, ds2, pl, e] with d = ds2*256 + pl*128 + pd
    pk_v = lambda w: np.ascontiguousarray(
        w.reshape(2, 2, 128, 512).transpose(2, 0, 1, 3))
    wv_p = pk_v(wv8)
    wvr_p = pk_v(wvr8)
    wo_p = s(np.asarray(Wo, np.float32).reshape(8, 64, 512).transpose(1, 0, 2))
    # bv folded into bo: cat(y_h + bv_h) @ Wo + bo = cat(y_h) @ Wo + bo'
    bo_p = s(np.asarray(bo, np.float32) +
             np.asarray(bv, np.float32).reshape(H * E) @ np.asarray(Wo, np.float32))
    return dict(wq=wq_p, wk=wk_p, wv=wv_p, wvr=wvr_p, wo=wo_p, bq=bq_p,
                bo=bo_p)


def kernel(x, Wq, bq, Wk, bk, Wv, bv, Wo, bo, **kw):
    global _NC
    x = np.asarray(x, np.float32)
    packed = _pack_weights(Wq, bq, Wk, bk, Wv, bv, Wo, bo)

    if _NC is None:
        _NC = build_program()

    in_maps = []
    for c in range(NCORES):
        b = c // 4
        q0 = (c % 4) * QCHUNK
        xb = np.roll(x[b], -q0, axis=0)  # queries at rows 0:1024
        import ml_dtypes
        xbT = xb.T  # [d, t]
        m = {"xt": np.ascontiguousarray(xbT.astype(ml_dtypes.bfloat16)),
             "x8": np.ascontiguousarray(
                 xbT.astype(ml_dtypes.float8_e4m3)
                 .reshape(2, 2, 128, S).transpose(2, 0, 1, 3))}
        m.update(packed)
        in_maps.append(m)
    res = run_bass_kernel_spmd(_NC, in_maps, core_ids=list(range(NCORES)))
    out = np.empty((B, S, D), np.float32)
    for c in range(NCORES):
        b = c // 4
        q0 = (c % 4) * QCHUNK
        out[b, q0:q0 + QCHUNK] = res.results[c]["out"]
    return out


# revision 9
# speedup vs baseline: 1.0213x; 1.0004x over previous
"""Multi-head attention Trainium2 kernel, 8-core SPMD (v6).

Problem: x[2,4096,512], 8 heads of 64; per-head QKV proj, softmax(QK^T/8)V,
concat, output proj.

Sharding: sequence-parallel, no collectives. Core c handles batch b=c//4 and
query rows [1024*(c%4), ...+1024). Each core computes K/V for the full 4096-row
sequence of its batch; x is host-rolled so local query rows sit at 0:1024.

v4 structure:
- bk dropped (softmax-invariant); bv folded into bo host-side.
- PSUM: shared 3-deep [128,2,512] ring for scores AND per-eg projection
  microtasks + 2 PV accumulators = 8 banks. Fine proj tasks retire ring
  slots quickly (convert ~0.6us vs exp ~1.1us), keeping recycling fast.
- projection emitted as per-eg microtasks (4 matmuls + 1 convert) popped
  between score pairs, so PE never runs >1us of projection while the exp
  engines starve.
- exp lanes near-alternate ACT/DVE (weighted 0.54 toward the faster ACT)
  so consecutive score slots retire on different engines; converts and
  drains are assigned by an ns-accurate load balancer.
- normalize: DVE reciprocal + Pool partition_broadcast + Pool multiply
  (no DRAM round-trip).
- group A ramps per availability; full-16 blocks skip the yacc merge.
- V projection in fp8 DoubleRow with a host-side weight-residual
  (Wv ~ W8 + fp8(Wv-W8)): 4 DR matmuls of contraction 256 instead of 4
  bf16 matmuls of contraction 128 -> half the PE time; error stays at
  the x-fp8-quantization level which the PV averaging washes out.
"""

import numpy as np

import concourse.bass as bass
from concourse import bacc
import concourse.mybir as mybir
import concourse.tile as tile
from concourse.bass_utils import run_bass_kernel_spmd

F32 = mybir.dt.float32
F32R = mybir.dt.float32r
BF16 = mybir.dt.bfloat16
FP8 = mybir.dt.float8e4
I8 = mybir.dt.int8
DR = mybir.MatmulPerfMode.DoubleRow
ADD = mybir.AluOpType.add
MULT = mybir.AluOpType.mult
EXPF = mybir.ActivationFunctionType.Exp
IDENT = mybir.ActivationFunctionType.Identity

B, S, D, H, E = 2, 4096, 512, 8, 64
NCORES = 8
QCHUNK = S // 4
TCH = 512
NPAIR = S // 256

A_SCH = 1.442695041      # 8*log2(e)*0.125
B_SCH = 55.632

A_CNT = [2, 2, 4, 4, 6, 8, 10, 12, 14, 16, 16, 16, 16, 16, 16, 16]
LAG = 3                  # pairs of exp->PV lag
PPOP = 2                 # pop 1 proj microtask every PPOP pairs
D_PRELOAD = 0.0          # initial DVE virtual load


def build_program():
    nc = bacc.Bacc()
    xt_d = nc.dram_tensor("xt", [D, S], BF16, kind="ExternalInput")
    wq_d = nc.dram_tensor("wq", [128, 4, 512], BF16, kind="ExternalInput")
    wk_d = nc.dram_tensor("wk", [128, 4, 512], BF16, kind="ExternalInput")
    x8_d = nc.dram_tensor("x8", [128, 2, 2, S], FP8, kind="ExternalInput")
    wv_d = nc.dram_tensor("wv", [128, 2, 2, 512], FP8, kind="ExternalInput")
    wvr_d = nc.dram_tensor("wvr", [128, 2, 2, 512], FP8, kind="ExternalInput")
    wo_d = nc.dram_tensor("wo", [64, 8, 512], F32R, kind="ExternalInput")
    bq_d = nc.dram_tensor("bq", [128, 4], F32, kind="ExternalInput")
    bo_d = nc.dram_tensor("bo", [512], F32, kind="ExternalInput")
    out_d = nc.dram_tensor("out", [QCHUNK, D], F32, kind="ExternalOutput")

    with tile.TileContext(nc) as tc:
        with (
            tc.tile_pool(name="const", bufs=1) as cpool,
            tc.tile_pool(name="work", bufs=6) as wpool,
            tc.tile_pool(name="misc", bufs=3) as mpool,
            tc.tile_pool(name="xtp", bufs=3) as xpool,
            tc.tile_pool(name="sring", bufs=6, space="PSUM") as spool,
            tc.tile_pool(name="pav", bufs=2, space="PSUM") as pavpool,
        ):
            wq_s = cpool.tile([128, 4, 512], BF16, tag="wq")
            wk_s = cpool.tile([128, 4, 512], BF16, tag="wk")
            x8_s = cpool.tile([128, 2, 2, S], FP8, tag="x8")
            wv_s = cpool.tile([128, 2, 2, 512], FP8, tag="wv")
            wvr_s = cpool.tile([128, 2, 2, 512], FP8, tag="wvr")
            wo_s = cpool.tile([64, 8, 512], F32R, tag="wo")
            bq_s = cpool.tile([128, 4], F32, tag="bq")
            bo_r = cpool.tile([128, 512], F32, tag="bor")

            kT8 = cpool.tile([128, 4, S], FP8, tag="kT8")
            qT8 = cpool.tile([128, 4, QCHUNK], FP8, tag="qT8")
            vA = cpool.tile([128, S // 128, H, E + 2], FP8, tag="vA")
            yacc = cpool.tile([65, 16, 512], F32, tag="yacc")
            yT2 = cpool.tile([64, H, QCHUNK], F32R, tag="yT2")

            nc.sync.dma_start(wk_s[:], wk_d[:])
            nc.gpsimd.memset(vA[:, :, :, E], 1.0)
            nc.gpsimd.memset(vA[:, :, :, E + 1], 0.0)

            # --- engine load balancing ------------------------------------
            load = {"A": 0.0, "D": D_PRELOAD}

            def pick():
                return "A" if load["A"] <= load["D"] else "D"

            def act_ns(free):
                return free * 0.8333 + 185.0

            def dve_ns(free):
                return free * 1.0417 + 125.0

            # --- projection microtasks ------------------------------------
            xstate = {}
            chunks_done = [0]

            def t_x(ch):
                def f():
                    xT = xpool.tile([128, 4, TCH], BF16, tag="xT")
                    for ds in range(4):
                        nc.sync.dma_start(
                            xT[:, ds, :],
                            xt_d[ds * 128:(ds + 1) * 128,
                                 ch * TCH:(ch + 1) * TCH])
                    xstate[ch] = xT
                return f

            def t_k(ch, eg):
                def f():
                    xT = xstate[ch]
                    Pt = spool.tile([128, 512], F32, tag="S")
                    for ds in range(4):
                        nc.tensor.matmul(
                            Pt[:], wk_s[:, ds, eg * 128:(eg + 1) * 128],
                            xT[:, ds, :], start=(ds == 0), stop=(ds == 3))
                    dst = kT8[:, eg, ch * TCH:(ch + 1) * TCH]
                    if pick() == "A":
                        nc.scalar.activation(dst, Pt[:], IDENT, scale=1.0)
                        load["A"] += act_ns(512)
                    else:
                        nc.vector.tensor_copy(dst, Pt[:])
                        load["D"] += dve_ns(512)
                return f

            def t_q(ch, eg):
                def f():
                    xT = xstate[ch]
                    Pt = spool.tile([128, 512], F32, tag="S")
                    for ds in range(4):
                        nc.tensor.matmul(
                            Pt[:], wq_s[:, ds, eg * 128:(eg + 1) * 128],
                            xT[:, ds, :], start=(ds == 0), stop=(ds == 3))
                    dst = qT8[:, eg, ch * TCH:(ch + 1) * TCH]
                    if pick() == "A":
                        nc.scalar.activation(dst, Pt[:], IDENT,
                                             bias=bq_s[:, eg:eg + 1],
                                             scale=1.0)
                        load["A"] += act_ns(512)
                    else:
                        nc.vector.tensor_scalar(dst, Pt[:],
                                                bq_s[:, eg:eg + 1], None, ADD)
                        load["D"] += dve_ns(512)
                return f

            def t_v(ch, ts):
                def f():
                    Pt = spool.tile([128, 512], F32, tag="S")
                    t0 = ch * TCH + ts * 128
                    step = 0
                    for wt in (wv_s, wvr_s):
                        for ds2 in range(2):
                            nc.tensor.matmul(
                                Pt[:], x8_s[:, ds2, :, t0:t0 + 128],
                                wt[:, ds2, :, :], start=(step == 0),
                                stop=(step == 3), perf_mode=DR)
                            step += 1
                    dst = vA[:, ch * 4 + ts, :, 0:E]
                    src = Pt[:].rearrange("p (h e) -> p h e", h=H)
                    if pick() == "A":
                        nc.scalar.activation(dst, src, IDENT, scale=1.0)
                        load["A"] += act_ns(512)
                    else:
                        nc.vector.tensor_copy(dst, src)
                        load["D"] += dve_ns(512)
                    if ts == 3:
                        chunks_done[0] = max(chunks_done[0], ch + 1)
                return f

            def chunk_tasks(ch):
                ts = [t_k(ch, 0), t_k(ch, 1)]
                if ch < 2:
                    ts += [t_q(ch, 0), t_q(ch, 1)]
                ts += [t_k(ch, 2), t_k(ch, 3)]
                if ch < 2:
                    ts += [t_q(ch, 2), t_q(ch, 3)]
                ts += [t_v(ch, 0), t_v(ch, 1), t_v(ch, 2), t_v(ch, 3)]
                return ts

            proj_queue = []
            pop_ctr = [0]

            def pop_proj(n=1):
                for _ in range(n):
                    if proj_queue:
                        proj_queue.pop(0)()

            def need_chunk(c):
                while chunks_done[0] <= c and proj_queue:
                    pop_proj(1)

            # --- attention block ------------------------------------------
            def emit_block(h, sc, pairs, mode):
                a0 = 32 * (h % 4)
                g0 = 2 * (h // 4)
                n = len(pairs)
                pav = pavpool.tile([128, 512], F32, tag="pav")
                pend = []

                def emit_pv(item):
                    j, tp, ptv = item
                    nc.tensor.matmul(
                        pav[0:66, :], vA[:, 2 * tp:2 * tp + 2, h, :], ptv,
                        start=(j == 0), stop=(j == n - 1), perf_mode=DR)

                for i in range(n):
                    tp = pairs[i]
                    need_chunk(tp // 2)
                    ptp = wpool.tile([128, 2, 512], I8, tag="ptD")
                    for kt in (0, 1):
                        tt = 2 * tp + kt
                        Sk = spool.tile([128, 512], F32, tag="S")
                        nc.tensor.matmul(
                            Sk[:],
                            kT8[a0:a0 + 32, g0:g0 + 2,
                                tt * 128:(tt + 1) * 128],
                            qT8[a0:a0 + 32, g0:g0 + 2,
                                sc * 512:(sc + 1) * 512],
                            start=True, stop=True, perf_mode=DR,
                            tile_position=(a0, 0))
                        if pick() == "A":
                            nc.scalar.activation(
                                ptp[:, kt, :].bitcast(FP8), Sk[:], EXPF,
                                scale=0.125)
                            load["A"] += act_ns(512)
                        else:
                            nc.vector.tensor_scalar(
                                ptp[:, kt, :], Sk[:], A_SCH, B_SCH, MULT,
                                ADD)
                            load["D"] += dve_ns(512)
                    ptv = ptp[:].bitcast(FP8)
                    pend.append((i, tp, ptv))
                    pop_ctr[0] += 1
                    if pop_ctr[0] >= PPOP:
                        pop_ctr[0] = 0
                        pop_proj(1)
                    while len(pend) > LAG:
                        emit_pv(pend.pop(0))
                for item in pend:
                    emit_pv(item)

                slot = sc * 8 + h
                if mode == "partA":
                    if pick() == "A":
                        nc.scalar.activation(yacc[:, slot, :], pav[0:65, :],
                                             IDENT, scale=1.0)
                        load["A"] += act_ns(512)
                    else:
                        nc.vector.tensor_copy(yacc[:, slot, :], pav[0:65, :])
                        load["D"] += dve_ns(512)
                    return
                if mode == "full":
                    tmp = mpool.tile([65, 512], F32, tag="tmp")
                    if pick() == "A":
                        nc.scalar.activation(tmp[:], pav[0:65, :], IDENT,
                                             scale=1.0)
                        load["A"] += act_ns(512)
                    else:
                        nc.vector.tensor_copy(tmp[:], pav[0:65, :])
                        load["D"] += dve_ns(512)
                else:  # "B"
                    tmp = mpool.tile([65, 512], F32, tag="tmp")
                    nc.vector.tensor_tensor(
                        out=tmp[:], in0=yacc[:, slot, :], in1=pav[0:65, :],
                        op=ADD)
                    load["D"] += dve_ns(512)
                rec = mpool.tile([1, 512], F32, tag="rec")
                nc.vector.reciprocal(rec[:], tmp[64:65, :])
                load["D"] += dve_ns(512)
                den = mpool.tile([64, 512], F32, tag="den")
                nc.gpsimd.partition_broadcast(den[:], rec[:])
                nc.gpsimd.tensor_tensor(
                    out=yT2[0:64, h, sc * 512:(sc + 1) * 512],
                    in0=tmp[0:64, :], in1=den[:], op=MULT)

            # --- output projection ----------------------------------------
            def phase3_task(sc, sta):
                def f():
                    for i in range(2):
                        st = sc * 4 + sta * 2 + i
                        po = pavpool.tile([128, 512], F32, tag="pav")
                        for h in range(H):
                            nc.tensor.matmul(
                                po[:],
                                yT2[0:64, h, st * 128:(st + 1) * 128],
                                wo_s[0:64, h, :], start=(h == 0),
                                stop=(h == 7))
                        o_s = mpool.tile([128, 512], F32, tag="osb")
                        nc.vector.tensor_tensor(out=o_s[:], in0=po[:],
                                                in1=bo_r[:], op=ADD)
                        load["D"] += dve_ns(512)
                        nc.sync.dma_start(out_d[st * 128:(st + 1) * 128, :],
                                          o_s[:])
                return f

            # --- emission --------------------------------------------------
            t_x(0)()
            t_k(0, 0)()
            nc.sync.dma_start(wq_s[:], wq_d[:])
            nc.sync.dma_start(bq_s[:], bq_d[:])
            t_k(0, 1)()
            t_q(0, 0)()
            t_q(0, 1)()
            nc.sync.dma_start(wv_s[:], wv_d[:])
            nc.sync.dma_start(wvr_s[:], wvr_d[:])
            nc.sync.dma_start(x8_s[:], x8_d[:])
            proj_queue.extend([t_k(0, 2), t_k(0, 3), t_q(0, 2), t_q(0, 3),
                               t_v(0, 0), t_v(0, 1), t_v(0, 2), t_v(0, 3)])
            for ch in range(1, 8):
                ts = chunk_tasks(ch)
                # promote the x DMA ahead of the previous chunk's tail
                ins = max(0, len(proj_queue) - 6)
                proj_queue.insert(ins, t_x(ch))
                proj_queue.extend(ts)

            blocks = [(sc, h) for sc in (0, 1) for h in range(H)]
            for bi, (sc, h) in enumerate(blocks):
                a = A_CNT[bi]
                mode = "full" if a == NPAIR else "partA"
                emit_block(h, sc, list(range(a)), mode)
                if bi == 11:
                    nc.sync.dma_start(
                        bo_r[:], bo_d[:].unsqueeze(0).to_broadcast((128, 512)))
                    nc.sync.dma_start(wo_s[:], wo_d[:])
                pop_proj(1)
            pop_proj(len(proj_queue))

            # group B: sc1 remainders first, then sc0 with p3(sc1) spread in
            for h in range(H):
                bi = 8 + h
                if A_CNT[bi] < NPAIR:
                    emit_block(h, 1, list(range(A_CNT[bi], NPAIR)), "B")
            p3s = [phase3_task(1, 0), phase3_task(1, 1)]
            for h in range(H):
                emit_block(h, 0, list(range(A_CNT[h], NPAIR)), "B")
                if p3s and h in (1, 3):
                    p3s.pop(0)()
            for t in p3s:
                t()
            phase3_task(0, 0)()
            phase3_task(0, 1)()
    nc.compile()
    return nc


_NC = None


def _pack_weights(Wq, bq, Wk, bk, Wv, bv, Wo, bo):
    import ml_dtypes
    s = lambda a: np.ascontiguousarray(np.asarray(a, np.float32))
    sb = lambda a: np.ascontiguousarray(
        np.asarray(a, np.float32).astype(ml_dtypes.bfloat16))
    # e-permutation for DoubleRow plane layout: column c = eg*128+p of the
    # stationary maps to head h = p//32 + 4*(eg//2), e = 32*(eg%2) + p%32
    p = np.arange(128)
    eg = np.arange(4)
    hh = p[None, :] // 32 + 4 * (eg[:, None] // 2)     # [4,128]
    ee = 32 * (eg[:, None] % 2) + p[None, :] % 32      # [4,128]

    def pack_qk(W):
        t = np.asarray(W, np.float32)[hh, :, ee]       # [4,128,512(d)]
        t = t.transpose(2, 0, 1)                       # [d, eg, p]
        t = t.reshape(4, 128, 4, 128)                  # [ds, pd, eg, p]
        return sb(t.transpose(1, 0, 2, 3).reshape(128, 4, 512))

    def pack_b(b):
        return s(np.asarray(b, np.float32)[hh, ee].T)  # [128,4]

    wq_p = pack_qk(Wq)
    wk_p = pack_qk(Wk)   # bk dropped: softmax-invariant
    bq_p = pack_b(bq)
    f8 = lambda a: np.asarray(a, np.float32).astype(ml_dtypes.float8_e4m3)
    wv_flat = np.transpose(np.asarray(Wv, np.float32), (1, 0, 2)).reshape(D, 512)
    wv8 = f8(wv_flat)
    wvr8 = f8(wv_flat - wv8.astype(np.float32))
    # [d, e] -> [pd, ds2, pl, e] with d = ds2*256 + pl*128 + pd
    pk_v = lambda w: np.ascontiguousarray(
        w.reshape(2, 2, 128, 512).transpose(2, 0, 1, 3))
    wv_p = pk_v(wv8)
    wvr_p = pk_v(wvr8)
    wo_p = s(np.asarray(Wo, np.float32).reshape(8, 64, 512).transpose(1, 0, 2))
    # bv folded into bo: cat(y_h + bv_h) @ Wo + bo = cat(y_h) @ Wo + bo'
    bo_p = s(np.asarray(bo, np.float32) +
             np.asarray(bv, np.float32).reshape(H * E) @ np.asarray(Wo, np.float32))
    return dict(wq=wq_p, wk=wk_p, wv=wv_p, wvr=wvr_p, wo=wo_p, bq=bq_p,
                bo=bo_p)


def kernel(x, Wq, bq, Wk, bk, Wv, bv, Wo, bo, **kw):
    global _NC
    x = np.asarray(x, np.float32)
    packed = _pack_weights(Wq, bq, Wk, bk, Wv, bv, Wo, bo)

    if _NC is None:
        _NC = build_program()

    in_maps = []
    for c in range(NCORES):
        b = c // 4
        q0 = (c % 4) * QCHUNK
        xb = np.roll(x[b], -q0, axis=0)  # queries at rows 0:1024
        import ml_dtypes
        xbT = xb.T  # [d, t]
        m = {"xt": np.ascontiguousarray(xbT.astype(ml_dtypes.bfloat16)),
             "x8": np.ascontiguousarray(
                 xbT.astype(ml_dtypes.float8_e4m3)
                 .reshape(2, 2, 128, S).transpose(2, 0, 1, 3))}
        m.update(packed)
        in_maps.append(m)
    res = run_bass_kernel_spmd(_NC, in_maps, core_ids=list(range(NCORES)))
    out = np.empty((B, S, D), np.float32)
    for c in range(NCORES):
        b = c // 4
        q0 = (c % 4) * QCHUNK
        out[b, q0:q0 + QCHUNK] = res.results[c]["out"]
    return out


# revision 10
# speedup vs baseline: 1.0227x; 1.0014x over previous
"""Multi-head attention Trainium2 kernel, 8-core SPMD (v6).

Problem: x[2,4096,512], 8 heads of 64; per-head QKV proj, softmax(QK^T/8)V,
concat, output proj.

Sharding: sequence-parallel, no collectives. Core c handles batch b=c//4 and
query rows [1024*(c%4), ...+1024). Each core computes K/V for the full 4096-row
sequence of its batch; x is host-rolled so local query rows sit at 0:1024.

v4 structure:
- bk dropped (softmax-invariant); bv folded into bo host-side.
- PSUM: shared 3-deep [128,2,512] ring for scores AND per-eg projection
  microtasks + 2 PV accumulators = 8 banks. Fine proj tasks retire ring
  slots quickly (convert ~0.6us vs exp ~1.1us), keeping recycling fast.
- projection emitted as per-eg microtasks (4 matmuls + 1 convert) popped
  between score pairs, so PE never runs >1us of projection while the exp
  engines starve.
- exp lanes near-alternate ACT/DVE (weighted 0.54 toward the faster ACT)
  so consecutive score slots retire on different engines; converts and
  drains are assigned by an ns-accurate load balancer.
- normalize: DVE reciprocal + Pool partition_broadcast + Pool multiply
  (no DRAM round-trip).
- group A ramps per availability; full-16 blocks skip the yacc merge.
- V projection in fp8 DoubleRow with a host-side weight-residual
  (Wv ~ W8 + fp8(Wv-W8)): 4 DR matmuls of contraction 256 instead of 4
  bf16 matmuls of contraction 128 -> half the PE time; error stays at
  the x-fp8-quantization level which the PV averaging washes out.
"""

import numpy as np

import concourse.bass as bass
from concourse import bacc
import concourse.mybir as mybir
import concourse.tile as tile
from concourse.bass_utils import run_bass_kernel_spmd

F32 = mybir.dt.float32
F32R = mybir.dt.float32r
BF16 = mybir.dt.bfloat16
FP8 = mybir.dt.float8e4
I8 = mybir.dt.int8
DR = mybir.MatmulPerfMode.DoubleRow
ADD = mybir.AluOpType.add
MULT = mybir.AluOpType.mult
EXPF = mybir.ActivationFunctionType.Exp
IDENT = mybir.ActivationFunctionType.Identity

B, S, D, H, E = 2, 4096, 512, 8, 64
NCORES = 8
QCHUNK = S // 4
TCH = 512
NPAIR = S // 256

A_SCH = 1.442695041      # 8*log2(e)*0.125
B_SCH = 55.632

A_CNT = [2, 2, 4, 4, 6, 8, 10, 12, 14, 16, 16, 16, 16, 16, 16, 16]
LAG = 3                  # pairs of exp->PV lag
PPOP = 2                 # pop 1 proj microtask every PPOP pairs
D_PRELOAD = 0.0          # initial DVE virtual load


def build_program():
    nc = bacc.Bacc()
    xt_d = nc.dram_tensor("xt", [D, S], BF16, kind="ExternalInput")
    wq_d = nc.dram_tensor("wq", [128, 4, 512], BF16, kind="ExternalInput")
    wk_d = nc.dram_tensor("wk", [128, 4, 512], BF16, kind="ExternalInput")
    x8_d = nc.dram_tensor("x8", [128, 2, 2, S], FP8, kind="ExternalInput")
    wv_d = nc.dram_tensor("wv", [128, 2, 2, 512], FP8, kind="ExternalInput")
    wvr_d = nc.dram_tensor("wvr", [128, 2, 2, 512], FP8, kind="ExternalInput")
    wo_d = nc.dram_tensor("wo", [64, 8, 512], F32R, kind="ExternalInput")
    bq_d = nc.dram_tensor("bq", [128, 4], F32, kind="ExternalInput")
    bo_d = nc.dram_tensor("bo", [512], F32, kind="ExternalInput")
    out_d = nc.dram_tensor("out", [QCHUNK, D], F32, kind="ExternalOutput")

    with tile.TileContext(nc) as tc:
        with (
            tc.tile_pool(name="const", bufs=1) as cpool,
            tc.tile_pool(name="work", bufs=6) as wpool,
            tc.tile_pool(name="misc", bufs=3) as mpool,
            tc.tile_pool(name="xtp", bufs=3) as xpool,
            tc.tile_pool(name="sring", bufs=6, space="PSUM") as spool,
            tc.tile_pool(name="pav", bufs=2, space="PSUM") as pavpool,
        ):
            wq_s = cpool.tile([128, 4, 512], BF16, tag="wq")
            wk_s = cpool.tile([128, 4, 512], BF16, tag="wk")
            x8_s = cpool.tile([128, 2, 2, S], FP8, tag="x8")
            wv_s = cpool.tile([128, 2, 2, 512], FP8, tag="wv")
            wvr_s = cpool.tile([128, 2, 2, 512], FP8, tag="wvr")
            wo_s = cpool.tile([64, 8, 512], F32R, tag="wo")
            bq_s = cpool.tile([128, 4], F32, tag="bq")
            bo_r = cpool.tile([128, 512], F32, tag="bor")

            kT8 = cpool.tile([128, 4, S], FP8, tag="kT8")
            qT8 = cpool.tile([128, 4, QCHUNK], FP8, tag="qT8")
            vA = cpool.tile([128, S // 128, H, E + 2], FP8, tag="vA")
            yacc = cpool.tile([65, 16, 512], F32, tag="yacc")
            yT2 = cpool.tile([64, H, QCHUNK], F32R, tag="yT2")

            nc.sync.dma_start(wk_s[:], wk_d[:])
            nc.gpsimd.memset(vA[:, :, :, E], 1.0)
            nc.gpsimd.memset(vA[:, :, :, E + 1], 0.0)

            # --- engine load balancing ------------------------------------
            load = {"A": 0.0, "D": D_PRELOAD}

            def pick():
                return "A" if load["A"] <= load["D"] else "D"

            def act_ns(free):
                return free * 0.8333 + 185.0

            def dve_ns(free):
                return free * 1.0417 + 125.0

            # --- projection microtasks ------------------------------------
            xstate = {}
            chunks_done = [0]

            def t_x(ch):
                def f():
                    xT = xpool.tile([128, 4, TCH], BF16, tag="xT")
                    for ds in range(4):
                        nc.sync.dma_start(
                            xT[:, ds, :],
                            xt_d[ds * 128:(ds + 1) * 128,
                                 ch * TCH:(ch + 1) * TCH])
                    xstate[ch] = xT
                return f

            def t_k(ch, eg):
                def f():
                    xT = xstate[ch]
                    Pt = spool.tile([128, 512], F32, tag="S")
                    for ds in range(4):
                        nc.tensor.matmul(
                            Pt[:], wk_s[:, ds, eg * 128:(eg + 1) * 128],
                            xT[:, ds, :], start=(ds == 0), stop=(ds == 3))
                    dst = kT8[:, eg, ch * TCH:(ch + 1) * TCH]
                    if pick() == "A":
                        nc.scalar.activation(dst, Pt[:], IDENT, scale=1.0)
                        load["A"] += act_ns(512)
                    else:
                        nc.vector.tensor_copy(dst, Pt[:])
                        load["D"] += dve_ns(512)
                return f

            def t_q(ch, eg):
                def f():
                    xT = xstate[ch]
                    Pt = spool.tile([128, 512], F32, tag="S")
                    for ds in range(4):
                        nc.tensor.matmul(
                            Pt[:], wq_s[:, ds, eg * 128:(eg + 1) * 128],
                            xT[:, ds, :], start=(ds == 0), stop=(ds == 3))
                    dst = qT8[:, eg, ch * TCH:(ch + 1) * TCH]
                    if pick() == "A":
                        nc.scalar.activation(dst, Pt[:], IDENT,
                                             bias=bq_s[:, eg:eg + 1],
                                             scale=1.0)
                        load["A"] += act_ns(512)
                    else:
                        nc.vector.tensor_scalar(dst, Pt[:],
                                                bq_s[:, eg:eg + 1], None, ADD)
                        load["D"] += dve_ns(512)
                return f

            def t_v(ch, ts):
                def f():
                    Pt = spool.tile([128, 512], F32, tag="S")
                    t0 = ch * TCH + ts * 128
                    step = 0
                    for wt in (wv_s, wvr_s):
                        for ds2 in range(2):
                            nc.tensor.matmul(
                                Pt[:], x8_s[:, ds2, :, t0:t0 + 128],
                                wt[:, ds2, :, :], start=(step == 0),
                                stop=(step == 3), perf_mode=DR)
                            step += 1
                    dst = vA[:, ch * 4 + ts, :, 0:E]
                    src = Pt[:].rearrange("p (h e) -> p h e", h=H)
                    if pick() == "A":
                        nc.scalar.activation(dst, src, IDENT, scale=1.0)
                        load["A"] += act_ns(512)
                    else:
                        nc.vector.tensor_copy(dst, src)
                        load["D"] += dve_ns(512)
                    if ts == 3:
                        chunks_done[0] = max(chunks_done[0], ch + 1)
                return f

            def chunk_tasks(ch):
                ts = [t_k(ch, 0), t_k(ch, 1)]
                if ch < 2:
                    ts += [t_q(ch, 0), t_q(ch, 1)]
                ts += [t_k(ch, 2), t_k(ch, 3)]
                if ch < 2:
                    ts += [t_q(ch, 2), t_q(ch, 3)]
                ts += [t_v(ch, 0), t_v(ch, 1), t_v(ch, 2), t_v(ch, 3)]
                return ts

            proj_queue = []
            pop_ctr = [0]

            def pop_proj(n=1):
                for _ in range(n):
                    if proj_queue:
                        proj_queue.pop(0)()

            def need_chunk(c):
                while chunks_done[0] <= c and proj_queue:
                    pop_proj(1)

            # --- attention block ------------------------------------------
            def emit_block(h, sc, pairs, mode):
                a0 = 32 * (h % 4)
                g0 = 2 * (h // 4)
                n = len(pairs)
                pav = pavpool.tile([128, 512], F32, tag="pav")
                pend = []

                def emit_pv(item):
                    j, tp, ptv = item
                    nc.tensor.matmul(
                        pav[0:66, :], vA[:, 2 * tp:2 * tp + 2, h, :], ptv,
                        start=(j == 0), stop=(j == n - 1), perf_mode=DR)

                for i in range(n):
                    tp = pairs[i]
                    need_chunk(tp // 2)
                    ptp = wpool.tile([128, 2, 512], I8, tag="ptD")
                    for kt in (0, 1):
                        tt = 2 * tp + kt
                        Sk = spool.tile([128, 512], F32, tag="S")
                        nc.tensor.matmul(
                            Sk[:],
                            kT8[a0:a0 + 32, g0:g0 + 2,
                                tt * 128:(tt + 1) * 128],
                            qT8[a0:a0 + 32, g0:g0 + 2,
                                sc * 512:(sc + 1) * 512],
                            start=True, stop=True, perf_mode=DR,
                            tile_position=(a0, 0))
                        if pick() == "A":
                            nc.scalar.activation(
                                ptp[:, kt, :].bitcast(FP8), Sk[:], EXPF,
                                scale=0.125)
                            load["A"] += act_ns(512)
                        else:
                            nc.vector.tensor_scalar(
                                ptp[:, kt, :], Sk[:], A_SCH, B_SCH, MULT,
                                ADD)
                            load["D"] += dve_ns(512)
                    ptv = ptp[:].bitcast(FP8)
                    pend.append((i, tp, ptv))
                    pop_ctr[0] += 1
                    if pop_ctr[0] >= PPOP:
                        pop_ctr[0] = 0
                        pop_proj(1)
                    while len(pend) > LAG:
                        emit_pv(pend.pop(0))
                for item in pend:
                    emit_pv(item)

                slot = sc * 8 + h
                if mode == "partA":
                    if pick() == "A":
                        nc.scalar.activation(yacc[:, slot, :], pav[0:65, :],
                                             IDENT, scale=1.0)
                        load["A"] += act_ns(512)
                    else:
                        nc.vector.tensor_copy(yacc[:, slot, :], pav[0:65, :])
                        load["D"] += dve_ns(512)
                    return
                if mode == "full":
                    tmp = mpool.tile([65, 512], F32, tag="tmp")
                    if pick() == "A":
                        nc.scalar.activation(tmp[:], pav[0:65, :], IDENT,
                                             scale=1.0)
                        load["A"] += act_ns(512)
                    else:
                        nc.vector.tensor_copy(tmp[:], pav[0:65, :])
                        load["D"] += dve_ns(512)
                else:  # "B"
                    tmp = mpool.tile([65, 512], F32, tag="tmp")
                    nc.vector.tensor_tensor(
                        out=tmp[:], in0=yacc[:, slot, :], in1=pav[0:65, :],
                        op=ADD)
                    load["D"] += dve_ns(512)
                rec = mpool.tile([1, 512], F32, tag="rec")
                nc.vector.reciprocal(rec[:], tmp[64:65, :])
                load["D"] += dve_ns(512)
                den = mpool.tile([64, 512], F32, tag="den")
                nc.gpsimd.partition_broadcast(den[:], rec[:])
                nc.gpsimd.tensor_tensor(
                    out=yT2[0:64, h, sc * 512:(sc + 1) * 512],
                    in0=tmp[0:64, :], in1=den[:], op=MULT)

            # --- output projection ----------------------------------------
            def phase3_task(sc, sta):
                def f():
                    for i in range(2):
                        st = sc * 4 + sta * 2 + i
                        po = pavpool.tile([128, 512], F32, tag="pav")
                        for h in range(H):
                            nc.tensor.matmul(
                                po[:],
                                yT2[0:64, h, st * 128:(st + 1) * 128],
                                wo_s[0:64, h, :], start=(h == 0),
                                stop=(h == 7))
                        o_s = mpool.tile([128, 512], F32, tag="osb")
                        nc.vector.tensor_tensor(out=o_s[:], in0=po[:],
                                                in1=bo_r[:], op=ADD)
                        load["D"] += dve_ns(512)
                        nc.sync.dma_start(out_d[st * 128:(st + 1) * 128, :],
                                          o_s[:])
                return f

            # --- emission --------------------------------------------------
            t_x(0)()
            t_k(0, 0)()
            nc.sync.dma_start(wq_s[:], wq_d[:])
            nc.sync.dma_start(bq_s[:], bq_d[:])
            t_k(0, 1)()
            t_q(0, 0)()
            t_q(0, 1)()
            nc.sync.dma_start(x8_s[:], x8_d[:])
            nc.sync.dma_start(wv_s[:], wv_d[:])
            nc.sync.dma_start(wvr_s[:], wvr_d[:])
            proj_queue.extend([t_k(0, 2), t_k(0, 3), t_q(0, 2), t_q(0, 3),
                               t_v(0, 0), t_v(0, 1), t_v(0, 2), t_v(0, 3)])
            for ch in range(1, 8):
                ts = chunk_tasks(ch)
                # promote the x DMA ahead of the previous chunk's tail
                ins = max(0, len(proj_queue) - 6)
                proj_queue.insert(ins, t_x(ch))
                proj_queue.extend(ts)

            blocks = [(sc, h) for sc in (0, 1) for h in range(H)]
            for bi, (sc, h) in enumerate(blocks):
                a = A_CNT[bi]
                mode = "full" if a == NPAIR else "partA"
                emit_block(h, sc, list(range(a)), mode)
                if bi == 11:
                    nc.sync.dma_start(
                        bo_r[:], bo_d[:].unsqueeze(0).to_broadcast((128, 512)))
                    nc.sync.dma_start(wo_s[:], wo_d[:])
                pop_proj(1)
            pop_proj(len(proj_queue))

            # group B: sc1 remainders first, then sc0 with p3(sc1) spread in
            for h in range(H):
                bi = 8 + h
                if A_CNT[bi] < NPAIR:
                    emit_block(h, 1, list(range(A_CNT[bi], NPAIR)), "B")
            p3s = [phase3_task(1, 0), phase3_task(1, 1)]
            for h in range(H):
                emit_block(h, 0, list(range(A_CNT[h], NPAIR)), "B")
                if p3s and h in (1, 3):
                    p3s.pop(0)()
            for t in p3s:
                t()
            phase3_task(0, 0)()
            phase3_task(0, 1)()
    nc.compile()
    return nc


_NC = None


def _pack_weights(Wq, bq, Wk, bk, Wv, bv, Wo, bo):
    import ml_dtypes
    s = lambda a: np.ascontiguousarray(np.asarray(a, np.float32))
    sb = lambda a: np.ascontiguousarray(
        np.asarray(a, np.float32).astype(ml_dtypes.bfloat16))
    # e-permutation for DoubleRow plane layout: column c = eg*128+p of the
    # stationary maps to head h = p//32 + 4*(eg//2), e = 32*(eg%2) + p%32
    p = np.arange(128)
    eg = np.arange(4)
    hh = p[None, :] // 32 + 4 * (eg[:, None] // 2)     # [4,128]
    ee = 32 * (eg[:, None] % 2) + p[None, :] % 32      # [4,128]

    def pack_qk(W):
        t = np.asarray(W, np.float32)[hh, :, ee]       # [4,128,512(d)]
        t = t.transpose(2, 0, 1)                       # [d, eg, p]
        t = t.reshape(4, 128, 4, 128)                  # [ds, pd, eg, p]
        return sb(t.transpose(1, 0, 2, 3).reshape(128, 4, 512))

    def pack_b(b):
        return s(np.asarray(b, np.float32)[hh, ee].T)  # [128,4]

    wq_p = pack_qk(Wq)
    wk_p = pack_qk(Wk)   # bk dropped: softmax-invariant
    bq_p = pack_b(bq)
    f8 = lambda a: np.asarray(a, np.float32).astype(ml_dtypes.float8_e4m3)
    wv_flat = np.transpose(np.asarray(Wv, np.float32), (1, 0, 2)).reshape(D, 512)
    wv8 = f8(wv_flat)
    wvr8 = f8(wv_flat - wv8.astype(np.float32))
    # [d, e] -> [pd, ds2, pl, e] with d = ds2*256 + pl*128 + pd
    pk_v = lambda w: np.ascontiguousarray(
        w.reshape(2, 2, 128, 512).transpose(2, 0, 1, 3))
    wv_p = pk_v(wv8)
    wvr_p = pk_v(wvr8)
    wo_p = s(np.asarray(Wo, np.float32).reshape(8, 64, 512).transpose(1, 0, 2))
    # bv folded into bo: cat(y_h + bv_h) @ Wo + bo = cat(y_h) @ Wo + bo'
    bo_p = s(np.asarray(bo, np.float32) +
             np.asarray(bv, np.float32).reshape(H * E) @ np.asarray(Wo, np.float32))
    return dict(wq=wq_p, wk=wk_p, wv=wv_p, wvr=wvr_p, wo=wo_p, bq=bq_p,
                bo=bo_p)


def kernel(x, Wq, bq, Wk, bk, Wv, bv, Wo, bo, **kw):
    global _NC
    x = np.asarray(x, np.float32)
    packed = _pack_weights(Wq, bq, Wk, bk, Wv, bv, Wo, bo)

    if _NC is None:
        _NC = build_program()

    in_maps = []
    for c in range(NCORES):
        b = c // 4
        q0 = (c % 4) * QCHUNK
        xb = np.roll(x[b], -q0, axis=0)  # queries at rows 0:1024
        import ml_dtypes
        xbT = xb.T  # [d, t]
        m = {"xt": np.ascontiguousarray(xbT.astype(ml_dtypes.bfloat16)),
             "x8": np.ascontiguousarray(
                 xbT.astype(ml_dtypes.float8_e4m3)
                 .reshape(2, 2, 128, S).transpose(2, 0, 1, 3))}
        m.update(packed)
        in_maps.append(m)
    res = run_bass_kernel_spmd(_NC, in_maps, core_ids=list(range(NCORES)))
    out = np.empty((B, S, D), np.float32)
    for c in range(NCORES):
        b = c // 4
        q0 = (c % 4) * QCHUNK
        out[b, q0:q0 + QCHUNK] = res.results[c]["out"]
    return out


# revision 11
# speedup vs baseline: 1.0278x; 1.0050x over previous
"""Multi-head attention Trainium2 kernel, 8-core SPMD (v6).

Problem: x[2,4096,512], 8 heads of 64; per-head QKV proj, softmax(QK^T/8)V,
concat, output proj.

Sharding: sequence-parallel, no collectives. Core c handles batch b=c//4 and
query rows [1024*(c%4), ...+1024). Each core computes K/V for the full 4096-row
sequence of its batch; x is host-rolled so local query rows sit at 0:1024.

v4 structure:
- bk dropped (softmax-invariant); bv folded into bo host-side.
- PSUM: shared 3-deep [128,2,512] ring for scores AND per-eg projection
  microtasks + 2 PV accumulators = 8 banks. Fine proj tasks retire ring
  slots quickly (convert ~0.6us vs exp ~1.1us), keeping recycling fast.
- projection emitted as per-eg microtasks (4 matmuls + 1 convert) popped
  between score pairs, so PE never runs >1us of projection while the exp
  engines starve.
- exp lanes near-alternate ACT/DVE (weighted 0.54 toward the faster ACT)
  so consecutive score slots retire on different engines; converts and
  drains are assigned by an ns-accurate load balancer.
- normalize: DVE reciprocal + Pool partition_broadcast + Pool multiply
  (no DRAM round-trip).
- group A ramps per availability; full-16 blocks skip the yacc merge.
- V projection in fp8 DoubleRow with a host-side weight-residual
  (Wv ~ W8 + fp8(Wv-W8)): 4 DR matmuls of contraction 256 instead of 4
  bf16 matmuls of contraction 128 -> half the PE time; error stays at
  the x-fp8-quantization level which the PV averaging washes out.
"""

import numpy as np

import concourse.bass as bass
from concourse import bacc
import concourse.mybir as mybir
import concourse.tile as tile
from concourse.bass_utils import run_bass_kernel_spmd

F32 = mybir.dt.float32
F32R = mybir.dt.float32r
BF16 = mybir.dt.bfloat16
FP8 = mybir.dt.float8e4
I8 = mybir.dt.int8
DR = mybir.MatmulPerfMode.DoubleRow
ADD = mybir.AluOpType.add
MULT = mybir.AluOpType.mult
EXPF = mybir.ActivationFunctionType.Exp
IDENT = mybir.ActivationFunctionType.Identity

B, S, D, H, E = 2, 4096, 512, 8, 64
NCORES = 8
QCHUNK = S // 4
TCH = 512
NPAIR = S // 256

A_SCH = 1.442695041      # 8*log2(e)*0.125
B_SCH = 55.632

A_CNT = [2, 2, 4, 4, 6, 8, 10, 12, 14, 16, 16, 16, 16, 16, 16, 16]
LAG = 3                  # pairs of exp->PV lag
PPOP = 2                 # pop 1 proj microtask every PPOP pairs
D_PRELOAD = 0.0          # initial DVE virtual load


def build_program():
    nc = bacc.Bacc()
    xt_d = nc.dram_tensor("xt", [D, S], BF16, kind="ExternalInput")
    wq_d = nc.dram_tensor("wq", [128, 4, 512], BF16, kind="ExternalInput")
    wk_d = nc.dram_tensor("wk", [128, 4, 512], BF16, kind="ExternalInput")
    x8_d = nc.dram_tensor("x8", [128, 2, 2, S], FP8, kind="ExternalInput")
    wv_d = nc.dram_tensor("wv", [128, 2, 2, 512], FP8, kind="ExternalInput")
    wvr_d = nc.dram_tensor("wvr", [128, 2, 2, 512], FP8, kind="ExternalInput")
    wo_d = nc.dram_tensor("wo", [64, 8, 512], F32R, kind="ExternalInput")
    bq_d = nc.dram_tensor("bq", [128, 4], F32, kind="ExternalInput")
    bo_d = nc.dram_tensor("bo", [512], F32, kind="ExternalInput")
    out_d = nc.dram_tensor("out", [QCHUNK, D], F32, kind="ExternalOutput")

    with tile.TileContext(nc) as tc:
        with (
            tc.tile_pool(name="const", bufs=1) as cpool,
            tc.tile_pool(name="work", bufs=6) as wpool,
            tc.tile_pool(name="misc", bufs=3) as mpool,
            tc.tile_pool(name="xtp", bufs=3) as xpool,
            tc.tile_pool(name="sring", bufs=6, space="PSUM") as spool,
            tc.tile_pool(name="pav", bufs=2, space="PSUM") as pavpool,
        ):
            wq_s = cpool.tile([128, 4, 512], BF16, tag="wq")
            wk_s = cpool.tile([128, 4, 512], BF16, tag="wk")
            x8_s = cpool.tile([128, 2, 2, S], FP8, tag="x8")
            wv_s = cpool.tile([128, 2, 2, 512], FP8, tag="wv")
            wvr_s = cpool.tile([128, 2, 2, 512], FP8, tag="wvr")
            wo_s = cpool.tile([64, 8, 512], F32R, tag="wo")
            bq_s = cpool.tile([128, 4], F32, tag="bq")
            bo_r = cpool.tile([128, 512], F32, tag="bor")

            kT8 = cpool.tile([128, 4, S], FP8, tag="kT8")
            qT8 = cpool.tile([128, 4, QCHUNK], FP8, tag="qT8")
            vA = cpool.tile([128, S // 128, H, E + 2], FP8, tag="vA")
            yacc = cpool.tile([65, 16, 512], F32, tag="yacc")
            yT2 = cpool.tile([64, H, QCHUNK], F32R, tag="yT2")

            nc.sync.dma_start(wk_s[:], wk_d[:])
            nc.gpsimd.memset(vA[:, :, :, E], 1.0)
            nc.gpsimd.memset(vA[:, :, :, E + 1], 0.0)

            # --- engine load balancing ------------------------------------
            load = {"A": 0.0, "D": D_PRELOAD}

            def pick():
                return "A" if load["A"] <= load["D"] else "D"

            def act_ns(free):
                return free * 0.8333 + 185.0

            def dve_ns(free):
                return free * 1.0417 + 125.0

            # --- projection microtasks ------------------------------------
            xstate = {}
            chunks_done = [0]

            def t_x(ch):
                def f():
                    xT = xpool.tile([128, 4, TCH], BF16, tag="xT")
                    for ds in range(4):
                        nc.sync.dma_start(
                            xT[:, ds, :],
                            xt_d[ds * 128:(ds + 1) * 128,
                                 ch * TCH:(ch + 1) * TCH])
                    xstate[ch] = xT
                return f

            def t_k(ch, eg):
                def f():
                    xT = xstate[ch]
                    Pt = spool.tile([128, 512], F32, tag="S")
                    for ds in range(4):
                        nc.tensor.matmul(
                            Pt[:], wk_s[:, ds, eg * 128:(eg + 1) * 128],
                            xT[:, ds, :], start=(ds == 0), stop=(ds == 3))
                    dst = kT8[:, eg, ch * TCH:(ch + 1) * TCH]
                    if pick() == "A":
                        nc.scalar.activation(dst, Pt[:], IDENT, scale=1.0)
                        load["A"] += act_ns(512)
                    else:
                        nc.vector.tensor_copy(dst, Pt[:])
                        load["D"] += dve_ns(512)
                return f

            def t_q(ch, eg):
                def f():
                    xT = xstate[ch]
                    Pt = spool.tile([128, 512], F32, tag="S")
                    for ds in range(4):
                        nc.tensor.matmul(
                            Pt[:], wq_s[:, ds, eg * 128:(eg + 1) * 128],
                            xT[:, ds, :], start=(ds == 0), stop=(ds == 3))
                    dst = qT8[:, eg, ch * TCH:(ch + 1) * TCH]
                    if pick() == "A":
                        nc.scalar.activation(dst, Pt[:], IDENT,
                                             bias=bq_s[:, eg:eg + 1],
                                             scale=1.0)
                        load["A"] += act_ns(512)
                    else:
                        nc.vector.tensor_scalar(dst, Pt[:],
                                                bq_s[:, eg:eg + 1], None, ADD)
                        load["D"] += dve_ns(512)
                return f

            def t_v(ch, ts):
                def f():
                    Pt = spool.tile([128, 512], F32, tag="S")
                    t0 = ch * TCH + ts * 128
                    step = 0
                    for wt in (wv_s, wvr_s):
                        for ds2 in range(2):
                            nc.tensor.matmul(
                                Pt[:], x8_s[:, ds2, :, t0:t0 + 128],
                                wt[:, ds2, :, :], start=(step == 0),
                                stop=(step == 3), perf_mode=DR)
                            step += 1
                    dst = vA[:, ch * 4 + ts, :, 0:E]
                    src = Pt[:].rearrange("p (h e) -> p h e", h=H)
                    if pick() == "A":
                        nc.scalar.activation(dst, src, IDENT, scale=1.0)
                        load["A"] += act_ns(512)
                    else:
                        nc.vector.tensor_copy(dst, src)
                        load["D"] += dve_ns(512)
                    if ts == 3:
                        chunks_done[0] = max(chunks_done[0], ch + 1)
                return f

            def chunk_tasks(ch):
                ts = [t_k(ch, 0), t_k(ch, 1)]
                if ch < 2:
                    ts += [t_q(ch, 0), t_q(ch, 1)]
                ts += [t_k(ch, 2), t_k(ch, 3)]
                if ch < 2:
                    ts += [t_q(ch, 2), t_q(ch, 3)]
                ts += [t_v(ch, 0), t_v(ch, 1), t_v(ch, 2), t_v(ch, 3)]
                return ts

            proj_queue = []
            pop_ctr = [0]

            def pop_proj(n=1):
                for _ in range(n):
                    if proj_queue:
                        proj_queue.pop(0)()

            def need_chunk(c):
                while chunks_done[0] <= c and proj_queue:
                    pop_proj(1)

            # --- attention block ------------------------------------------
            def emit_block(h, sc, pairs, mode):
                a0 = 32 * (h % 4)
                g0 = 2 * (h // 4)
                n = len(pairs)
                pav = pavpool.tile([128, 512], F32, tag="pav")
                pend = []

                def emit_pv(item):
                    j, tp, ptv = item
                    nc.tensor.matmul(
                        pav[0:66, :], vA[:, 2 * tp:2 * tp + 2, h, :], ptv,
                        start=(j == 0), stop=(j == n - 1), perf_mode=DR)

                for i in range(n):
                    tp = pairs[i]
                    need_chunk(tp // 2)
                    ptp = wpool.tile([128, 2, 512], I8, tag="ptD")
                    for kt in (0, 1):
                        tt = 2 * tp + kt
                        Sk = spool.tile([128, 512], F32, tag="S")
                        nc.tensor.matmul(
                            Sk[:],
                            kT8[a0:a0 + 32, g0:g0 + 2,
                                tt * 128:(tt + 1) * 128],
                            qT8[a0:a0 + 32, g0:g0 + 2,
                                sc * 512:(sc + 1) * 512],
                            start=True, stop=True, perf_mode=DR,
                            tile_position=(a0, 0))
                        if pick() == "A":
                            nc.scalar.activation(
                                ptp[:, kt, :].bitcast(FP8), Sk[:], EXPF,
                                scale=0.125)
                            load["A"] += act_ns(512)
                        else:
                            nc.vector.tensor_scalar(
                                ptp[:, kt, :], Sk[:], A_SCH, B_SCH, MULT,
                                ADD)
                            load["D"] += dve_ns(512)
                    ptv = ptp[:].bitcast(FP8)
                    pend.append((i, tp, ptv))
                    pop_ctr[0] += 1
                    if pop_ctr[0] >= PPOP:
                        pop_ctr[0] = 0
                        pop_proj(1)
                    while len(pend) > LAG:
                        emit_pv(pend.pop(0))
                for item in pend:
                    emit_pv(item)

                slot = sc * 8 + h
                if mode == "partA":
                    if pick() == "A":
                        nc.scalar.activation(yacc[:, slot, :], pav[0:65, :],
                                             IDENT, scale=1.0)
                        load["A"] += act_ns(512)
                    else:
                        nc.vector.tensor_copy(yacc[:, slot, :], pav[0:65, :])
                        load["D"] += dve_ns(512)
                    return
                if mode == "full":
                    tmp = mpool.tile([65, 512], F32, tag="tmp")
                    if pick() == "A":
                        nc.scalar.activation(tmp[:], pav[0:65, :], IDENT,
                                             scale=1.0)
                        load["A"] += act_ns(512)
                    else:
                        nc.vector.tensor_copy(tmp[:], pav[0:65, :])
                        load["D"] += dve_ns(512)
                else:  # "B"
                    tmp = mpool.tile([65, 512], F32, tag="tmp")
                    nc.vector.tensor_tensor(
                        out=tmp[:], in0=yacc[:, slot, :], in1=pav[0:65, :],
                        op=ADD)
                    load["D"] += dve_ns(512)
                rec = mpool.tile([1, 512], F32, tag="rec")
                nc.vector.reciprocal(rec[:], tmp[64:65, :])
                load["D"] += dve_ns(512)
                den = mpool.tile([64, 512], F32, tag="den")
                nc.gpsimd.partition_broadcast(den[:], rec[:])
                nc.gpsimd.tensor_tensor(
                    out=yT2[0:64, h, sc * 512:(sc + 1) * 512],
                    in0=tmp[0:64, :], in1=den[:], op=MULT)

            # --- output projection ----------------------------------------
            def phase3_task(sc, sta):
                def f():
                    for i in range(2):
                        st = sc * 4 + sta * 2 + i
                        po = pavpool.tile([128, 512], F32, tag="pav")
                        for h in range(H):
                            nc.tensor.matmul(
                                po[:],
                                yT2[0:64, h, st * 128:(st + 1) * 128],
                                wo_s[0:64, h, :], start=(h == 0),
                                stop=(h == 7))
                        o_s = mpool.tile([128, 512], F32, tag="osb")
                        nc.vector.tensor_tensor(out=o_s[:], in0=po[:],
                                                in1=bo_r[:], op=ADD)
                        load["D"] += dve_ns(512)
                        nc.sync.dma_start(out_d[st * 128:(st + 1) * 128, :],
                                          o_s[:])
                return f

            # --- emission --------------------------------------------------
            t_x(0)()
            t_k(0, 0)()
            nc.sync.dma_start(wq_s[:], wq_d[:])
            nc.sync.dma_start(bq_s[:], bq_d[:])
            t_k(0, 1)()
            t_q(0, 0)()
            t_q(0, 1)()
            nc.sync.dma_start(x8_s[:], x8_d[:])
            nc.sync.dma_start(wv_s[:], wv_d[:])
            nc.sync.dma_start(wvr_s[:], wvr_d[:])
            proj_queue.extend([t_k(0, 2), t_k(0, 3), t_q(0, 2), t_q(0, 3),
                               t_v(0, 0), t_v(0, 1), t_v(0, 2), t_v(0, 3)])
            for ch in range(1, 8):
                ts = chunk_tasks(ch)
                # promote the x DMA ahead of the previous chunk's tail
                ins = max(0, len(proj_queue) - 8)
                proj_queue.insert(ins, t_x(ch))
                proj_queue.extend(ts)

            blocks = [(sc, h) for sc in (0, 1) for h in range(H)]
            for bi, (sc, h) in enumerate(blocks):
                a = A_CNT[bi]
                mode = "full" if a == NPAIR else "partA"
                emit_block(h, sc, list(range(a)), mode)
                if bi == 11:
                    nc.sync.dma_start(
                        bo_r[:], bo_d[:].unsqueeze(0).to_broadcast((128, 512)))
                    nc.sync.dma_start(wo_s[:], wo_d[:])
                pop_proj(1)
            pop_proj(len(proj_queue))

            # group B: sc1 remainders first, then sc0 with p3(sc1) spread in
            for h in range(H):
                bi = 8 + h
                if A_CNT[bi] < NPAIR:
                    emit_block(h, 1, list(range(A_CNT[bi], NPAIR)), "B")
            p3s = [phase3_task(1, 0), phase3_task(1, 1)]
            for h in range(H):
                emit_block(h, 0, list(range(A_CNT[h], NPAIR)), "B")
                if p3s and h in (1, 3):
                    p3s.pop(0)()
            for t in p3s:
                t()
            phase3_task(0, 0)()
            phase3_task(0, 1)()
    nc.compile()
    return nc


_NC = None


def _pack_weights(Wq, bq, Wk, bk, Wv, bv, Wo, bo):
    import ml_dtypes
    s = lambda a: np.ascontiguousarray(np.asarray(a, np.float32))
    sb = lambda a: np.ascontiguousarray(
        np.asarray(a, np.float32).astype(ml_dtypes.bfloat16))
    # e-permutation for DoubleRow plane layout: column c = eg*128+p of the
    # stationary maps to head h = p//32 + 4*(eg//2), e = 32*(eg%2) + p%32
    p = np.arange(128)
    eg = np.arange(4)
    hh = p[None, :] // 32 + 4 * (eg[:, None] // 2)     # [4,128]
    ee = 32 * (eg[:, None] % 2) + p[None, :] % 32      # [4,128]

    def pack_qk(W):
        t = np.asarray(W, np.float32)[hh, :, ee]       # [4,128,512(d)]
        t = t.transpose(2, 0, 1)                       # [d, eg, p]
        t = t.reshape(4, 128, 4, 128)                  # [ds, pd, eg, p]
        return sb(t.transpose(1, 0, 2, 3).reshape(128, 4, 512))

    def pack_b(b):
        return s(np.asarray(b, np.float32)[hh, ee].T)  # [128,4]

    wq_p = pack_qk(Wq)
    wk_p = pack_qk(Wk)   # bk dropped: softmax-invariant
    bq_p = pack_b(bq)
    f8 = lambda a: np.asarray(a, np.float32).astype(ml_dtypes.float8_e4m3)
    wv_flat = np.transpose(np.asarray(Wv, np.float32), (1, 0, 2)).reshape(D, 512)
    wv8 = f8(wv_flat)
    wvr8 = f8(wv_flat - wv8.astype(np.float32))
    # [d, e] -> [pd, ds2, pl, e] with d = ds2*256 + pl*128 + pd
    pk_v = lambda w: np.ascontiguousarray(
        w.reshape(2, 2, 128, 512).transpose(2, 0, 1, 3))
    wv_p = pk_v(wv8)
    wvr_p = pk_v(wvr8)
    wo_p = s(np.asarray(Wo, np.float32).reshape(8, 64, 512).transpose(1, 0, 2))
    # bv folded into bo: cat(y_h + bv_h) @ Wo + bo = cat(y_h) @ Wo + bo'
    bo_p = s(np.asarray(bo, np.float32) +
             np.asarray(bv, np.float32).reshape(H * E) @ np.asarray(Wo, np.float32))
    return dict(wq=wq_p, wk=wk_p, wv=wv_p, wvr=wvr_p, wo=wo_p, bq=bq_p,
                bo=bo_p)


def kernel(x, Wq, bq, Wk, bk, Wv, bv, Wo, bo, **kw):
    global _NC
    x = np.asarray(x, np.float32)
    packed = _pack_weights(Wq, bq, Wk, bk, Wv, bv, Wo, bo)

    if _NC is None:
        _NC = build_program()

    in_maps = []
    for c in range(NCORES):
        b = c // 4
        q0 = (c % 4) * QCHUNK
        xb = np.roll(x[b], -q0, axis=0)  # queries at rows 0:1024
        import ml_dtypes
        xbT = xb.T  # [d, t]
        m = {"xt": np.ascontiguousarray(xbT.astype(ml_dtypes.bfloat16)),
             "x8": np.ascontiguousarray(
                 xbT.astype(ml_dtypes.float8_e4m3)
                 .reshape(2, 2, 128, S).transpose(2, 0, 1, 3))}
        m.update(packed)
        in_maps.append(m)
    res = run_bass_kernel_spmd(_NC, in_maps, core_ids=list(range(NCORES)))
    out = np.empty((B, S, D), np.float32)
    for c in range(NCORES):
        b = c // 4
        q0 = (c % 4) * QCHUNK
        out[b, q0:q0 + QCHUNK] = res.results[c]["out"]
    return out
